# revision 22
# baseline (speedup 1.0000x reference)
"""Contextual loss kernel for Trainium2 (Bass/Tile), 8 NeuronCores.

Reference computation (per batch b, B=4, C=128, N=64*64=4096):
  mean_y[c] = spatial mean of feature_y
  fx,fy centered by mean_y; columns L2-normalized over channels
  S[n,m]    = <fxn[:,n], fyn[:,m]>           (cosine similarity)
  d = 1-S;  d_norm = d / (min_m d + 1e-3);  w = exp((1-d_norm)/h);  A = w/sum_m w
  CX[b] = mean_n max_m A;  loss = -log(CX)

Per-row identity used on device (with Smax = max_m S, c = 1/(h*(1-Smax+eps))):
  max_m A = 1 / sum_m exp(c*(S[m]-Smax))

Sharding: 8 cores = 4 batches x 2 row-halves. Each core gets its half of
feature_x's rows ([2048,128]) plus the full feature_y ([4096,128]) of its
batch, computes sum_rows 1/r locally; host combines and takes -log.
Row/column order within a core is permuted (contiguous-DMA layout); all
reductions over S are permutation-invariant so the result is unchanged.

Head: inputs land via two HWDGE queues with one large descriptor per
partition; the y mean accumulates per chunk as it arrives. Preprocessing
(center on gpsimd/DVE, square+sqrt on ACT, norm reduce+recip on DVE,
scale, bf16 PE transpose, 2x-mode PSUM->SBUF casts) is software-pipelined
per 8-tile group so the main loop starts as soon as the last y group lands.

Main loop per 128-row block (all bf16 matmuls, two passes, recompute):
pass 1 fills four 2-bank PSUM quarters and DVE row-maxes each (the row max
must complete before any exp: c = 1/(h*(1+eps-Smax)) scales the exponent);
a tiny chain (gpsimd+DVE) builds c and bias = -Smax*c; pass 2 recomputes
the same panels one block behind and ACT exp's each quarter in place with
accumulate (row sums). Recompute keeps PE streaming: PSUM (8 banks) cannot
hold two blocks, and every engine reading PSUM f32 is capped at 1
elem/cycle/lane, so S is cheaper to recompute than to stage elsewhere.
Steady state is ACT-bound (~5.3us/block): exp is 1 elem/cycle/lane at
1.2 GHz and only the scalar engine has exp.
"""

import numpy as np

import concourse.bacc as bacc
import concourse.bass as bass
import concourse.tile as tile
from concourse import masks, mybir
from concourse.bass_utils import run_bass_kernel_spmd

F32 = mybir.dt.float32
BF16 = mybir.dt.bfloat16
F32R = mybir.dt.float32r
AF = mybir.ActivationFunctionType

B = 4
C = 128
N = 4096          # spatial positions per batch
ROWS = N // 2     # rows of S per core (x-half)
P = 128           # partitions
NYT = N // P      # 32 y tiles
NXT = ROWS // P   # 16 x tiles
NT = NYT + NXT    # 48 tiles to preprocess
CHUNK = 512       # matmul free dim (one PSUM bank)
QUART = 1024      # columns per PSUM quarter (2 banks)
NQ = N // QUART   # 4 quarters per row block
NRB = ROWS // P   # 16 row blocks per core

H_PARAM = 0.1
EPS_MIN = 0.001
EPS_NORM = 1e-10

# bf16 matmul: 1 cyc/row at any PE p-state (f32r needs full 2.4GHz clock to
# hit 1 cyc/row; at the mid p-state it is no faster but the PE rarely ramps
# in this kernel). bf16 input quantization adds ~1e-3 loss error (tol 2e-2).
MM_DT = mybir.dt.bfloat16


def build_nc():
    nc = bacc.Bacc(None)
    fx = nc.declare_dram_parameter("fx", [ROWS, C], F32, isOutput=False)
    fy = nc.declare_dram_parameter("fy", [N, C], F32, isOutput=False)
    part = nc.declare_dram_parameter("part", [P, 1], F32, isOutput=True)

    # Partition p takes CONTIGUOUS spatial rows [k*p, k*(p+1)): one large
    # descriptor per partition per DMA instead of 512B strided lines. This
    # permutes S's rows/columns vs the reference, but every reduction over
    # them (row max, row sum, final sum over rows) is permutation-invariant.
    fy_t = fy.rearrange("(p i) c -> p i c", p=P)   # [128, 32, 128]
    fx_t = fx.rearrange("(p i) c -> p i c", p=P)   # [128, 16, 128]

    with tile.TileContext(nc) as tc:
        with (
            tc.tile_pool(name="singles", bufs=1) as singles,
            tc.tile_pool(name="raw", bufs=1) as raw,
            tc.tile_pool(name="tmats", bufs=1) as tmats,
            tc.tile_pool(name="stat", bufs=3) as stat,
            tc.tile_pool(name="scratch", bufs=2) as scratch,
        ):
            # ---- constants ----
            identity_bf = singles.tile([P, P], BF16)
            masks.make_identity(nc, identity_bf[:])
            ones_col = singles.tile([P, 1], F32)
            nc.vector.memset(ones_col[:], 1.0)
            ones_row = singles.tile([1, P], F32)
            nc.vector.memset(ones_row[:], 1.0)

            ns_all = singles.tile([P, NT], F32)     # squared norms per tile
            inv_all = singles.tile([P, NT], F32)    # 1/(norm+eps)
            invr_all = singles.tile([P, NRB], F32)  # per-rowblock 1/r
            mean_sb = singles.tile([1, C], F32)
            mean_bc = singles.tile([P, C], F32)
            colsum = singles.tile([P, C], F32)      # per-partition y colsums

            # ---- load inputs; y on the sync DGE queue, x on scalar's ----
            # Partial column sums per 4-tile chunk start as soon as each
            # chunk lands, so the mean is ready right after the last chunk.
            ysp = raw.tile([P, NYT, C], F32)   # y, spatial-major tiles
            xsp = raw.tile([P, NXT, C], F32)
            ycb = raw.tile([P, NYT, C], BF16)  # centered+normalized, bf16
            xcb = raw.tile([P, NXT, C], BF16)
            colsum_p = singles.tile([P, 4, C], F32)
            # y split across both HWDGE queues so the mean unblocks sooner;
            # x follows on the scalar queue (not needed until after mean).
            for j in range(4):
                eng = nc.sync if j % 2 == 0 else nc.scalar
                eng.dma_start(
                    out=ysp[:, j * 8:(j + 1) * 8, :],
                    in_=fy_t[:, j * 8:(j + 1) * 8, :],
                )
                nc.vector.reduce_sum(
                    colsum_p[:, j, :],
                    ysp[:, j * 8:(j + 1) * 8, :].rearrange("p i c -> p c i"),
                    axis=mybir.AxisListType.X,
                )
            for j in range(2):
                nc.scalar.dma_start(
                    out=xsp[:, j * 8:(j + 1) * 8, :],
                    in_=fx_t[:, j * 8:(j + 1) * 8, :],
                )
            nc.vector.reduce_sum(
                colsum[:], colsum_p[:].rearrange("p j c -> p c j"),
                axis=mybir.AxisListType.X,
            )  # combine the 4 partials
            # PE: sum over partitions (ones^T @ colsum), scale by 1/N,
            # broadcast back over partitions via K=1 matmul
            with tc.tile_pool(name="ps_mean", bufs=1,
                              space=bass.MemorySpace.PSUM) as ps_mean_pool:
                ps_mean = ps_mean_pool.tile([1, C], F32)
                nc.tensor.matmul(ps_mean[:], ones_col[:], colsum[:],
                                 start=True, stop=True)
                nc.scalar.mul(mean_sb[:], ps_mean[:], 1.0 / N)
            with tc.tile_pool(name="ps_bc", bufs=1,
                              space=bass.MemorySpace.PSUM) as ps_bc_pool:
                ps_bc = ps_bc_pool.tile([P, C], F32)
                nc.tensor.matmul(ps_bc[:], ones_row[:], mean_sb[:],
                                 start=True, stop=True)
                nc.vector.tensor_copy(mean_bc[:], ps_bc[:])

            mean_g = mean_bc[:].rearrange("p (u c) -> p u c", u=1)

            # ---- per-group pipeline: center -> norms -> scale -> transpose
            # x first (its tiles gate every pass-1 matmul), then y groups in
            # quarter order. x's center/scale run on DVE so gpsimd (the
            # serial bottleneck of this phase) only carries the y groups.
            # Everything downstream of centering is bf16: transposes run at
            # 1 cyc/row and PSUM->SBUF copies hit the 2x packed DVE mode.
            ytc = [tmats.tile([P, CHUNK], MM_DT, tag=f"ytc{j}", name=f"ytc{j}")
                   for j in range(N // CHUNK)]     # y: [C, m] chunks
            xt = tmats.tile([P, ROWS], MM_DT)      # x: [C, n]

            GRP = 8
            std_all = singles.tile([P, NT], F32)
            # (src f32 view, centered bf16 view, tile base, is_x)
            groups = []
            for g in range(NXT // GRP):
                groups.append((xsp[:, g * GRP:(g + 1) * GRP, :],
                               xcb[:, g * GRP:(g + 1) * GRP, :],
                               NYT + g * GRP, True))
            for g in range(NYT // GRP):
                groups.append((ysp[:, g * GRP:(g + 1) * GRP, :],
                               ycb[:, g * GRP:(g + 1) * GRP, :],
                               g * GRP, False))

            def grp_eng(t0, is_x):
                # x center/scale on DVE, y on gpsimd: gpsimd is the serial
                # bottleneck of this phase, DVE carries everything else
                return nc.vector if is_x else nc.gpsimd

            def stage_center(src, cb, t0, is_x):
                eng = grp_eng(t0, is_x)
                eng.tensor_sub(cb, src, mean_g.broadcast_to([P, GRP, C]))
                sq = scratch.tile([P, GRP, C], F32, tag="sq")
                nc.scalar.activation(out=sq[:], in_=cb, func=AF.Square)
                nc.vector.reduce_sum(ns_all[:, t0:t0 + GRP], sq[:],
                                     axis=mybir.AxisListType.X)
                # inv = 1/sqrt(ns); the reference's +1e-10 is far below
                # bf16 resolution of these O(10) norms
                nc.scalar.activation(std_all[:, t0:t0 + GRP],
                                     ns_all[:, t0:t0 + GRP], AF.Sqrt)
                nc.vector.reciprocal(inv_all[:, t0:t0 + GRP],
                                     std_all[:, t0:t0 + GRP])

            def stage_scale(ps_tr_pool, src, cb, t0, is_x):
                eng = grp_eng(t0, is_x)
                ig = inv_all[:, t0:t0 + GRP].rearrange("p (t u) -> p t u", u=1)
                eng.tensor_mul(cb, cb, ig.broadcast_to([P, GRP, C]))
                for h in range(2):             # 2 batches of 4 tiles
                    pst = ps_tr_pool.tile([P, 4 * P], BF16, tag="pst")
                    for k in range(4):
                        nc.tensor.transpose(
                            pst[:, k * P:(k + 1) * P],
                            cb[:, 4 * h + k, :], identity_bf[:])
                    # split the PSUM->SBUF copies between DVE and the (idle
                    # during preproc) scalar engine
                    if is_x:
                        x0 = (t0 - NYT + 4 * h) * P
                        dst = xt[:, x0:x0 + 4 * P]
                    else:
                        dst = ytc[t0 // 4 + h][:]
                    if h == 0:
                        nc.vector.tensor_copy(dst, pst[:])
                    else:
                        nc.scalar.copy(dst, pst[:])

            # Software-pipelined issue, centering two groups ahead of the
            # scale, so gpsimd's mul(g) never head-blocks sub(g+1) while g's
            # norm chain (ACT square -> DVE reduce -> ACT sqrt -> DVE recip)
            # is in flight.
            with tc.tile_pool(name="ps_tr", bufs=4,
                              space=bass.MemorySpace.PSUM) as ps_tr_pool:
                stage_center(*groups[0])
                stage_center(*groups[1])
                for i, g in enumerate(groups):
                    stage_scale(ps_tr_pool, *g)
                    if i + 2 < len(groups):
                        stage_center(*groups[i + 2])

            # ---- main loop: two interleaved passes with recompute ----
            # Pass 1 (PE+DVE): matmul panels -> row-max, PSUM freed at max.
            # Pass 2 (PE+ACT): recompute the same panels -> exp in place with
            # accumulate. The second matmul is free (PE has slack) and the
            # recomputed S is bitwise identical. Passes run one block apart
            # in disjoint PSUM halves, so no engine waits on another's phase.
            cc_all = singles.tile([P, NRB], F32)   # 1/(H*(1+eps-Smax))
            nb_all = singles.tile([P, NRB], F32)   # -Smax*c
            r_all = singles.tile([P, NRB], F32)
            rq_all = singles.tile([P, NRB, NQ], F32)  # per-quarter exp sums

            def pass1(rb, pool):
                lhs = xt[:, rb * P:(rb + 1) * P]
                mxq = stat.tile([P, NQ], F32, tag="mxq", name="mxq")
                for q in range(NQ):
                    ps = pool.tile([P, QUART], F32, tag="p1", name="ps1")
                    for j in range(2):
                        nc.tensor.matmul(
                            ps[:, j * CHUNK:(j + 1) * CHUNK],
                            lhs, ytc[2 * q + j][:], start=True, stop=True)
                    nc.vector.reduce_max(mxq[:, q:q + 1], ps[:],
                                         axis=mybir.AxisListType.X)
                mx = stat.tile([P, 1], F32, tag="mx", name="mx")
                nc.vector.reduce_max(mx[:], mxq[:],
                                     axis=mybir.AxisListType.X)
                # scalar chain on gpsimd (idle during the main loop) except
                # the reciprocal, which only DVE has at full accuracy
                hd = stat.tile([P, 1], F32, tag="hd", name="hd")
                nc.gpsimd.tensor_scalar(
                    out=hd[:], in0=mx[:], scalar1=1.0 + EPS_MIN,
                    scalar2=-H_PARAM, op0=mybir.AluOpType.subtract,
                    op1=mybir.AluOpType.mult)
                nc.vector.reciprocal(cc_all[:, rb:rb + 1], hd[:])
                nc.gpsimd.tensor_scalar(
                    out=nb_all[:, rb:rb + 1], in0=mx[:],
                    scalar1=cc_all[:, rb:rb + 1], scalar2=-1.0,
                    op0=mybir.AluOpType.mult, op1=mybir.AluOpType.mult)

            def pass2(rb, pool):
                lhs = xt[:, rb * P:(rb + 1) * P]
                for q in range(NQ):
                    ps = pool.tile([P, QUART], F32, tag="p2", name="ps2")
                    for j in range(2):
                        nc.tensor.matmul(
                            ps[:, j * CHUNK:(j + 1) * CHUNK],
                            lhs, ytc[2 * q + j][:], start=True, stop=True)
                    nc.scalar.activation(
                        out=ps[:], in_=ps[:], func=AF.Exp,
                        bias=nb_all[:, rb:rb + 1], scale=cc_all[:, rb:rb + 1],
                        accum_out=rq_all[:, rb, q:q + 1])

            with (
                tc.tile_pool(name="ps_p1", bufs=2,
                             space=bass.MemorySpace.PSUM) as pool1,
                tc.tile_pool(name="ps_p2", bufs=2,
                             space=bass.MemorySpace.PSUM) as pool2,
            ):
                for rb in range(NRB + 1):
                    # issue pass2 first: its matmuls feed the ACT exp (the
                    # steady-state bottleneck), so they get PE queue priority
                    if rb >= 1:
                        pass2(rb - 1, pool2)
                    if rb < NRB:
                        pass1(rb, pool1)

            # ---- 1/r, reduce row contributions, write out ----
            nc.vector.reduce_sum(r_all[:], rq_all[:],
                                 axis=mybir.AxisListType.X)
            nc.vector.reciprocal(invr_all[:], r_all[:])
            part_sb = singles.tile([P, 1], F32)
            nc.vector.reduce_sum(part_sb[:], invr_all[:],
                                 axis=mybir.AxisListType.X)
            nc.scalar.dma_start(out=part[:], in_=part_sb[:])

    nc.compile()
    return nc


_NC_CACHE = None


def _get_nc():
    global _NC_CACHE
    if _NC_CACHE is None:
        _NC_CACHE = build_nc()
    return _NC_CACHE


def _in_maps(feature_x, feature_y):
    fx = np.ascontiguousarray(
        np.asarray(feature_x, dtype=np.float32).reshape(B, N, C))
    fy = np.ascontiguousarray(
        np.asarray(feature_y, dtype=np.float32).reshape(B, N, C))
    maps = []
    for core in range(8):
        b, h = divmod(core, 2)
        maps.append({
            "fx": np.ascontiguousarray(fx[b, h * ROWS:(h + 1) * ROWS, :]),
            "fy": fy[b],
        })
    return maps


def _combine(results):
    sums = [float(np.asarray(r["part"], dtype=np.float64).sum())
            for r in results]
    loss = np.empty(B, dtype=np.float64)
    for b in range(B):
        cx = (sums[2 * b] + sums[2 * b + 1]) / N
        loss[b] = -np.log(cx)
    return loss.astype(np.float32)


def kernel(feature_x, feature_y):
    nc = _get_nc()
    res = run_bass_kernel_spmd(nc, _in_maps(feature_x, feature_y),
                               core_ids=list(range(8)))
    return _combine(res.results)


def kernel_traced(feature_x, feature_y, **kwargs):
    """Like kernel() but with tracing; returns (loss, BassKernelResults)."""
    nc = _get_nc()
    res = run_bass_kernel_spmd(nc, _in_maps(feature_x, feature_y),
                               core_ids=list(range(8)), trace=True, **kwargs)
    return _combine(res.results), res



# revision 23
# speedup vs baseline: 1.0091x; 1.0091x over previous
"""Contextual loss kernel for Trainium2 (Bass/Tile), 8 NeuronCores.

Reference computation (per batch b, B=4, C=128, N=64*64=4096):
  mean_y[c] = spatial mean of feature_y
  fx,fy centered by mean_y; columns L2-normalized over channels
  S[n,m]    = <fxn[:,n], fyn[:,m]>           (cosine similarity)
  d = 1-S;  d_norm = d / (min_m d + 1e-3);  w = exp((1-d_norm)/h);  A = w/sum_m w
  CX[b] = mean_n max_m A;  loss = -log(CX)

Per-row identity used on device (with Smax = max_m S, c = 1/(h*(1-Smax+eps))):
  max_m A = 1 / sum_m exp(c*(S[m]-Smax))

Sharding: 8 cores = 4 batches x 2 row-halves. Each core gets its half of
feature_x's rows ([2048,128]) plus the full feature_y ([4096,128]) of its
batch, computes sum_rows 1/r locally; host combines and takes -log.
Row/column order within a core is permuted (contiguous-DMA layout); all
reductions over S are permutation-invariant so the result is unchanged.

Head: inputs land via two HWDGE queues with one large descriptor per
partition; the y mean accumulates per chunk as it arrives. Preprocessing
(center on gpsimd/DVE, square+sqrt on ACT, norm reduce+recip on DVE,
scale, bf16 PE transpose, 2x-mode PSUM->SBUF casts) is software-pipelined
per 8-tile group so the main loop starts as soon as the last y group lands.

Main loop per 128-row block (all bf16 matmuls, two passes, recompute):
pass 1 fills four 2-bank PSUM quarters and DVE row-maxes each (the row max
must complete before any exp: c = 1/(h*(1+eps-Smax)) scales the exponent);
a tiny chain (gpsimd+DVE) builds c and bias = -Smax*c; pass 2 recomputes
the same panels one block behind and ACT exp's each quarter in place with
accumulate (row sums). Recompute keeps PE streaming: PSUM (8 banks) cannot
hold two blocks, and every engine reading PSUM f32 is capped at 1
elem/cycle/lane, so S is cheaper to recompute than to stage elsewhere.
Steady state is ACT-bound (~5.3us/block): exp is 1 elem/cycle/lane at
1.2 GHz and only the scalar engine has exp.
"""

import numpy as np

import concourse.bacc as bacc
import concourse.bass as bass
import concourse.tile as tile
from concourse import masks, mybir
from concourse.bass_utils import run_bass_kernel_spmd

F32 = mybir.dt.float32
BF16 = mybir.dt.bfloat16
F32R = mybir.dt.float32r
AF = mybir.ActivationFunctionType

B = 4
C = 128
N = 4096          # spatial positions per batch
ROWS = N // 2     # rows of S per core (x-half)
P = 128           # partitions
NYT = N // P      # 32 y tiles
NXT = ROWS // P   # 16 x tiles
NT = NYT + NXT    # 48 tiles to preprocess
CHUNK = 512       # matmul free dim (one PSUM bank)
QUART = 1024      # columns per PSUM quarter (2 banks)
NQ = N // QUART   # 4 quarters per row block
NRB = ROWS // P   # 16 row blocks per core

H_PARAM = 0.1
EPS_MIN = 0.001
EPS_NORM = 1e-10

# bf16 matmul: 1 cyc/row at any PE p-state (f32r needs full 2.4GHz clock to
# hit 1 cyc/row; at the mid p-state it is no faster but the PE rarely ramps
# in this kernel). bf16 input quantization adds ~1e-3 loss error (tol 2e-2).
MM_DT = mybir.dt.bfloat16


def build_nc():
    nc = bacc.Bacc(None)
    fx = nc.declare_dram_parameter("fx", [ROWS, C], F32, isOutput=False)
    fy = nc.declare_dram_parameter("fy", [N, C], F32, isOutput=False)
    part = nc.declare_dram_parameter("part", [P, 1], F32, isOutput=True)

    # Partition p takes CONTIGUOUS spatial rows [k*p, k*(p+1)): one large
    # descriptor per partition per DMA instead of 512B strided lines. This
    # permutes S's rows/columns vs the reference, but every reduction over
    # them (row max, row sum, final sum over rows) is permutation-invariant.
    fy_t = fy.rearrange("(p i) c -> p i c", p=P)   # [128, 32, 128]
    fx_t = fx.rearrange("(p i) c -> p i c", p=P)   # [128, 16, 128]

    with tile.TileContext(nc) as tc:
        with (
            tc.tile_pool(name="singles", bufs=1) as singles,
            tc.tile_pool(name="raw", bufs=1) as raw,
            tc.tile_pool(name="tmats", bufs=1) as tmats,
            tc.tile_pool(name="stat", bufs=3) as stat,
            tc.tile_pool(name="scratch", bufs=2) as scratch,
        ):
            # ---- constants ----
            identity_bf = singles.tile([P, P], BF16)
            masks.make_identity(nc, identity_bf[:])
            ones_col = singles.tile([P, 1], F32)
            nc.vector.memset(ones_col[:], 1.0)
            ones_row = singles.tile([1, P], F32)
            nc.vector.memset(ones_row[:], 1.0)

            ns_all = singles.tile([P, NT], F32)     # squared norms per tile
            inv_all = singles.tile([P, NT], F32)    # 1/(norm+eps)
            invr_all = singles.tile([P, NRB], F32)  # per-rowblock 1/r
            mean_sb = singles.tile([1, C], F32)
            mean_bc = singles.tile([P, C], F32)
            colsum = singles.tile([P, C], F32)      # per-partition y colsums

            # ---- load inputs; y on the sync DGE queue, x on scalar's ----
            # Partial column sums per 4-tile chunk start as soon as each
            # chunk lands, so the mean is ready right after the last chunk.
            ysp = raw.tile([P, NYT, C], F32)   # y, spatial-major tiles
            xsp = raw.tile([P, NXT, C], F32)
            ycb = raw.tile([P, NYT, C], BF16)  # centered+normalized, bf16
            xcb = raw.tile([P, NXT, C], BF16)
            colsum_p = singles.tile([P, 8, C], F32)
            # y split across both HWDGE queues so the mean unblocks sooner;
            # x follows on the scalar queue (not needed until after mean).
            for j in range(8):
                eng = nc.sync if j % 2 == 0 else nc.scalar
                eng.dma_start(
                    out=ysp[:, j * 4:(j + 1) * 4, :],
                    in_=fy_t[:, j * 4:(j + 1) * 4, :],
                )
                nc.vector.reduce_sum(
                    colsum_p[:, j, :],
                    ysp[:, j * 4:(j + 1) * 4, :].rearrange("p i c -> p c i"),
                    axis=mybir.AxisListType.X,
                )
            for j in range(2):
                nc.scalar.dma_start(
                    out=xsp[:, j * 8:(j + 1) * 8, :],
                    in_=fx_t[:, j * 8:(j + 1) * 8, :],
                )
            nc.vector.reduce_sum(
                colsum[:], colsum_p[:].rearrange("p j c -> p c j"),
                axis=mybir.AxisListType.X,
            )  # combine the 4 partials
            # PE: sum over partitions (ones^T @ colsum), scale by 1/N,
            # broadcast back over partitions via K=1 matmul
            with tc.tile_pool(name="ps_mean", bufs=1,
                              space=bass.MemorySpace.PSUM) as ps_mean_pool:
                ps_mean = ps_mean_pool.tile([1, C], F32)
                nc.tensor.matmul(ps_mean[:], ones_col[:], colsum[:],
                                 start=True, stop=True)
                nc.scalar.mul(mean_sb[:], ps_mean[:], 1.0 / N)
            with tc.tile_pool(name="ps_bc", bufs=1,
                              space=bass.MemorySpace.PSUM) as ps_bc_pool:
                ps_bc = ps_bc_pool.tile([P, C], F32)
                nc.tensor.matmul(ps_bc[:], ones_row[:], mean_sb[:],
                                 start=True, stop=True)
                nc.vector.tensor_copy(mean_bc[:], ps_bc[:])

            mean_g = mean_bc[:].rearrange("p (u c) -> p u c", u=1)

            # ---- per-group pipeline: center -> norms -> scale -> transpose
            # x first (its tiles gate every pass-1 matmul), then y groups in
            # quarter order. x's center/scale run on DVE so gpsimd (the
            # serial bottleneck of this phase) only carries the y groups.
            # Everything downstream of centering is bf16: transposes run at
            # 1 cyc/row and PSUM->SBUF copies hit the 2x packed DVE mode.
            ytc = [tmats.tile([P, CHUNK], MM_DT, tag=f"ytc{j}", name=f"ytc{j}")
                   for j in range(N // CHUNK)]     # y: [C, m] chunks
            xt = tmats.tile([P, ROWS], MM_DT)      # x: [C, n]

            GRP = 8
            std_all = singles.tile([P, NT], F32)
            # (src f32 view, centered bf16 view, tile base, is_x)
            groups = []
            for g in range(NXT // GRP):
                groups.append((xsp[:, g * GRP:(g + 1) * GRP, :],
                               xcb[:, g * GRP:(g + 1) * GRP, :],
                               NYT + g * GRP, True))
            for g in range(NYT // GRP):
                groups.append((ysp[:, g * GRP:(g + 1) * GRP, :],
                               ycb[:, g * GRP:(g + 1) * GRP, :],
                               g * GRP, False))

            def grp_eng(t0, is_x):
                # x center/scale on DVE, y on gpsimd: gpsimd is the serial
                # bottleneck of this phase, DVE carries everything else
                return nc.vector if is_x else nc.gpsimd

            def stage_center(src, cb, t0, is_x):
                eng = grp_eng(t0, is_x)
                eng.tensor_sub(cb, src, mean_g.broadcast_to([P, GRP, C]))
                sq = scratch.tile([P, GRP, C], F32, tag="sq")
                nc.scalar.activation(out=sq[:], in_=cb, func=AF.Square)
                nc.vector.reduce_sum(ns_all[:, t0:t0 + GRP], sq[:],
                                     axis=mybir.AxisListType.X)
                # inv = 1/sqrt(ns); the reference's +1e-10 is far below
                # bf16 resolution of these O(10) norms
                nc.scalar.activation(std_all[:, t0:t0 + GRP],
                                     ns_all[:, t0:t0 + GRP], AF.Sqrt)
                nc.vector.reciprocal(inv_all[:, t0:t0 + GRP],
                                     std_all[:, t0:t0 + GRP])

            def stage_scale(ps_tr_pool, src, cb, t0, is_x):
                eng = grp_eng(t0, is_x)
                ig = inv_all[:, t0:t0 + GRP].rearrange("p (t u) -> p t u", u=1)
                eng.tensor_mul(cb, cb, ig.broadcast_to([P, GRP, C]))
                for h in range(2):             # 2 batches of 4 tiles
                    pst = ps_tr_pool.tile([P, 4 * P], BF16, tag="pst")
                    for k in range(4):
                        nc.tensor.transpose(
                            pst[:, k * P:(k + 1) * P],
                            cb[:, 4 * h + k, :], identity_bf[:])
                    # split the PSUM->SBUF copies between DVE and the (idle
                    # during preproc) scalar engine
                    if is_x:
                        x0 = (t0 - NYT + 4 * h) * P
                        dst = xt[:, x0:x0 + 4 * P]
                    else:
                        dst = ytc[t0 // 4 + h][:]
                    if h == 0:
                        nc.vector.tensor_copy(dst, pst[:])
                    else:
                        nc.scalar.copy(dst, pst[:])

            # Software-pipelined issue, centering two groups ahead of the
            # scale, so gpsimd's mul(g) never head-blocks sub(g+1) while g's
            # norm chain (ACT square -> DVE reduce -> ACT sqrt -> DVE recip)
            # is in flight.
            with tc.tile_pool(name="ps_tr", bufs=4,
                              space=bass.MemorySpace.PSUM) as ps_tr_pool:
                stage_center(*groups[0])
                stage_center(*groups[1])
                for i, g in enumerate(groups):
                    stage_scale(ps_tr_pool, *g)
                    if i + 2 < len(groups):
                        stage_center(*groups[i + 2])

            # ---- main loop: two interleaved passes with recompute ----
            # Pass 1 (PE+DVE): matmul panels -> row-max, PSUM freed at max.
            # Pass 2 (PE+ACT): recompute the same panels -> exp in place with
            # accumulate. The second matmul is free (PE has slack) and the
            # recomputed S is bitwise identical. Passes run one block apart
            # in disjoint PSUM halves, so no engine waits on another's phase.
            cc_all = singles.tile([P, NRB], F32)   # 1/(H*(1+eps-Smax))
            nb_all = singles.tile([P, NRB], F32)   # -Smax*c
            r_all = singles.tile([P, NRB], F32)
            rq_all = singles.tile([P, NRB, NQ], F32)  # per-quarter exp sums

            def pass1(rb, pool):
                lhs = xt[:, rb * P:(rb + 1) * P]
                mxq = stat.tile([P, NQ], F32, tag="mxq", name="mxq")
                for q in range(NQ):
                    ps = pool.tile([P, QUART], F32, tag="p1", name="ps1")
                    for j in range(2):
                        nc.tensor.matmul(
                            ps[:, j * CHUNK:(j + 1) * CHUNK],
                            lhs, ytc[2 * q + j][:], start=True, stop=True)
                    nc.vector.reduce_max(mxq[:, q:q + 1], ps[:],
                                         axis=mybir.AxisListType.X)
                mx = stat.tile([P, 1], F32, tag="mx", name="mx")
                nc.vector.reduce_max(mx[:], mxq[:],
                                     axis=mybir.AxisListType.X)
                # scalar chain on gpsimd (idle during the main loop) except
                # the reciprocal, which only DVE has at full accuracy
                hd = stat.tile([P, 1], F32, tag="hd", name="hd")
                nc.gpsimd.tensor_scalar(
                    out=hd[:], in0=mx[:], scalar1=1.0 + EPS_MIN,
                    scalar2=-H_PARAM, op0=mybir.AluOpType.subtract,
                    op1=mybir.AluOpType.mult)
                nc.vector.reciprocal(cc_all[:, rb:rb + 1], hd[:])
                nc.gpsimd.tensor_scalar(
                    out=nb_all[:, rb:rb + 1], in0=mx[:],
                    scalar1=cc_all[:, rb:rb + 1], scalar2=-1.0,
                    op0=mybir.AluOpType.mult, op1=mybir.AluOpType.mult)

            def pass2(rb, pool):
                lhs = xt[:, rb * P:(rb + 1) * P]
                for q in range(NQ):
                    ps = pool.tile([P, QUART], F32, tag="p2", name="ps2")
                    for j in range(2):
                        nc.tensor.matmul(
                            ps[:, j * CHUNK:(j + 1) * CHUNK],
                            lhs, ytc[2 * q + j][:], start=True, stop=True)
                    nc.scalar.activation(
                        out=ps[:], in_=ps[:], func=AF.Exp,
                        bias=nb_all[:, rb:rb + 1], scale=cc_all[:, rb:rb + 1],
                        accum_out=rq_all[:, rb, q:q + 1])

            with (
                tc.tile_pool(name="ps_p1", bufs=2,
                             space=bass.MemorySpace.PSUM) as pool1,
                tc.tile_pool(name="ps_p2", bufs=2,
                             space=bass.MemorySpace.PSUM) as pool2,
            ):
                for rb in range(NRB + 1):
                    # issue pass2 first: its matmuls feed the ACT exp (the
                    # steady-state bottleneck), so they get PE queue priority
                    if rb >= 1:
                        pass2(rb - 1, pool2)
                    if rb < NRB:
                        pass1(rb, pool1)

            # ---- 1/r, reduce row contributions, write out ----
            nc.vector.reduce_sum(r_all[:], rq_all[:],
                                 axis=mybir.AxisListType.X)
            nc.vector.reciprocal(invr_all[:], r_all[:])
            part_sb = singles.tile([P, 1], F32)
            nc.vector.reduce_sum(part_sb[:], invr_all[:],
                                 axis=mybir.AxisListType.X)
            nc.scalar.dma_start(out=part[:], in_=part_sb[:])

    nc.compile()
    return nc


_NC_CACHE = None


def _get_nc():
    global _NC_CACHE
    if _NC_CACHE is None:
        _NC_CACHE = build_nc()
    return _NC_CACHE


def _in_maps(feature_x, feature_y):
    fx = np.ascontiguousarray(
        np.asarray(feature_x, dtype=np.float32).reshape(B, N, C))
    fy = np.ascontiguousarray(
        np.asarray(feature_y, dtype=np.float32).reshape(B, N, C))
    maps = []
    for core in range(8):
        b, h = divmod(core, 2)
        maps.append({
            "fx": np.ascontiguousarray(fx[b, h * ROWS:(h + 1) * ROWS, :]),
            "fy": fy[b],
        })
    return maps


def _combine(results):
    sums = [float(np.asarray(r["part"], dtype=np.float64).sum())
            for r in results]
    loss = np.empty(B, dtype=np.float64)
    for b in range(B):
        cx = (sums[2 * b] + sums[2 * b + 1]) / N
        loss[b] = -np.log(cx)
    return loss.astype(np.float32)


def kernel(feature_x, feature_y):
    nc = _get_nc()
    res = run_bass_kernel_spmd(nc, _in_maps(feature_x, feature_y),
                               core_ids=list(range(8)))
    return _combine(res.results)


def kernel_traced(feature_x, feature_y, **kwargs):
    """Like kernel() but with tracing; returns (loss, BassKernelResults)."""
    nc = _get_nc()
    res = run_bass_kernel_spmd(nc, _in_maps(feature_x, feature_y),
                               core_ids=list(range(8)), trace=True, **kwargs)
    return _combine(res.results), res



# revision 25
# speedup vs baseline: 1.0268x; 1.0175x over previous
"""Contextual loss kernel for Trainium2 (Bass/Tile), 8 NeuronCores.

Reference computation (per batch b, B=4, C=128, N=64*64=4096):
  mean_y[c] = spatial mean of feature_y
  fx,fy centered by mean_y; columns L2-normalized over channels
  S[n,m]    = <fxn[:,n], fyn[:,m]>           (cosine similarity)
  d = 1-S;  d_norm = d / (min_m d + 1e-3);  w = exp((1-d_norm)/h);  A = w/sum_m w
  CX[b] = mean_n max_m A;  loss = -log(CX)

Per-row identity used on device (with Smax = max_m S, c = 1/(h*(1-Smax+eps))):
  max_m A = 1 / sum_m exp(c*(S[m]-Smax))

Sharding: 8 cores = 4 batches x 2 row-halves. Each core gets its half of
feature_x's rows ([2048,128]) plus the full feature_y ([4096,128]) of its
batch, computes sum_rows 1/r locally; host combines and takes -log.
Row/column order within a core is permuted (contiguous-DMA layout); all
reductions over S are permutation-invariant so the result is unchanged.

Head: inputs land via two HWDGE queues with one large descriptor per
partition; the y mean accumulates per chunk as it arrives. Preprocessing
(center on gpsimd/DVE, square+sqrt on ACT, norm reduce+recip on DVE,
scale, bf16 PE transpose, 2x-mode PSUM->SBUF casts) is software-pipelined
per 8-tile group so the main loop starts as soon as the last y group lands.

Main loop per 128-row block (all bf16 matmuls, two passes, recompute):
pass 1 fills four 2-bank PSUM quarters and DVE row-maxes each (the row max
must complete before any exp: c = 1/(h*(1+eps-Smax)) scales the exponent);
a tiny chain (gpsimd+DVE) builds c and bias = -Smax*c; pass 2 recomputes
the same panels one block behind and ACT exp's each quarter in place with
accumulate (row sums). Recompute keeps PE streaming: PSUM (8 banks) cannot
hold two blocks, and every engine reading PSUM f32 is capped at 1
elem/cycle/lane, so S is cheaper to recompute than to stage elsewhere.
Steady state is ACT-bound (~5.3us/block): exp is 1 elem/cycle/lane at
1.2 GHz and only the scalar engine has exp.
"""

import numpy as np

import concourse.bacc as bacc
import concourse.bass as bass
import concourse.tile as tile
from concourse import masks, mybir
from concourse.bass_utils import run_bass_kernel_spmd

F32 = mybir.dt.float32
BF16 = mybir.dt.bfloat16
F32R = mybir.dt.float32r
AF = mybir.ActivationFunctionType

B = 4
C = 128
N = 4096          # spatial positions per batch
ROWS = N // 2     # rows of S per core (x-half)
P = 128           # partitions
NYT = N // P      # 32 y tiles
NXT = ROWS // P   # 16 x tiles
NT = NYT + NXT    # 48 tiles to preprocess
CHUNK = 512       # matmul free dim (one PSUM bank)
QUART = 1024      # columns per PSUM quarter (2 banks)
NQ = N // QUART   # 4 quarters per row block
NRB = ROWS // P   # 16 row blocks per core

H_PARAM = 0.1
EPS_MIN = 0.001
EPS_NORM = 1e-10

# bf16 matmul: 1 cyc/row at any PE p-state (f32r needs full 2.4GHz clock to
# hit 1 cyc/row; at the mid p-state it is no faster but the PE rarely ramps
# in this kernel). bf16 input quantization adds ~1e-3 loss error (tol 2e-2).
MM_DT = mybir.dt.bfloat16


def build_nc():
    nc = bacc.Bacc(None)
    fx = nc.declare_dram_parameter("fx", [ROWS, C], F32, isOutput=False)
    fy = nc.declare_dram_parameter("fy", [N, C], F32, isOutput=False)
    part = nc.declare_dram_parameter("part", [P, 1], F32, isOutput=True)

    # Partition p takes CONTIGUOUS spatial rows [k*p, k*(p+1)): one large
    # descriptor per partition per DMA instead of 512B strided lines. This
    # permutes S's rows/columns vs the reference, but every reduction over
    # them (row max, row sum, final sum over rows) is permutation-invariant.
    fy_t = fy.rearrange("(p i) c -> p i c", p=P)   # [128, 32, 128]
    fx_t = fx.rearrange("(p i) c -> p i c", p=P)   # [128, 16, 128]

    with tile.TileContext(nc) as tc:
        with (
            tc.tile_pool(name="singles", bufs=1) as singles,
            tc.tile_pool(name="raw", bufs=1) as raw,
            tc.tile_pool(name="tmats", bufs=1) as tmats,
            tc.tile_pool(name="stat", bufs=3) as stat,
            tc.tile_pool(name="scratch", bufs=2) as scratch,
        ):
            # ---- constants ----
            identity_bf = singles.tile([P, P], BF16)
            masks.make_identity(nc, identity_bf[:])
            ones_col = singles.tile([P, 1], F32)
            nc.vector.memset(ones_col[:], 1.0)
            ones_row = singles.tile([1, P], F32)
            nc.vector.memset(ones_row[:], 1.0)

            ns_all = singles.tile([P, NT], F32)     # squared norms per tile
            inv_all = singles.tile([P, NT], F32)    # 1/(norm+eps)
            invr_all = singles.tile([P, NRB], F32)  # per-rowblock 1/r
            mean_sb = singles.tile([1, C], F32)
            mean_bc = singles.tile([P, C], F32)
            colsum = singles.tile([P, C], F32)      # per-partition y colsums

            # ---- load inputs; y on the sync DGE queue, x on scalar's ----
            # Partial column sums per 4-tile chunk start as soon as each
            # chunk lands, so the mean is ready right after the last chunk.
            ysp = raw.tile([P, NYT, C], F32)   # y, spatial-major tiles
            xsp = raw.tile([P, NXT, C], F32)
            ycb = raw.tile([P, NYT, C], BF16)  # centered+normalized, bf16
            xcb = raw.tile([P, NXT, C], BF16)
            colsum_p = singles.tile([P, 8, C], F32)
            # y split across both HWDGE queues so the mean unblocks sooner;
            # x follows on the scalar queue (not needed until after mean).
            for j in range(8):
                eng = nc.sync if j % 2 == 0 else nc.scalar
                eng.dma_start(
                    out=ysp[:, j * 4:(j + 1) * 4, :],
                    in_=fy_t[:, j * 4:(j + 1) * 4, :],
                )
                nc.vector.reduce_sum(
                    colsum_p[:, j, :],
                    ysp[:, j * 4:(j + 1) * 4, :].rearrange("p i c -> p c i"),
                    axis=mybir.AxisListType.X,
                )
            for j in range(2):
                nc.scalar.dma_start(
                    out=xsp[:, j * 8:(j + 1) * 8, :],
                    in_=fx_t[:, j * 8:(j + 1) * 8, :],
                )
            nc.vector.reduce_sum(
                colsum[:], colsum_p[:].rearrange("p j c -> p c j"),
                axis=mybir.AxisListType.X,
            )  # combine the 4 partials
            # PE: sum over partitions (ones^T @ colsum), scale by 1/N,
            # broadcast back over partitions via K=1 matmul
            with tc.tile_pool(name="ps_mean", bufs=1,
                              space=bass.MemorySpace.PSUM) as ps_mean_pool:
                ps_mean = ps_mean_pool.tile([1, C], F32)
                nc.tensor.matmul(ps_mean[:], ones_col[:], colsum[:],
                                 start=True, stop=True)
                nc.scalar.mul(mean_sb[:], ps_mean[:], 1.0 / N)
            with tc.tile_pool(name="ps_bc", bufs=1,
                              space=bass.MemorySpace.PSUM) as ps_bc_pool:
                ps_bc = ps_bc_pool.tile([P, C], F32)
                nc.tensor.matmul(ps_bc[:], ones_row[:], mean_sb[:],
                                 start=True, stop=True)
                nc.vector.tensor_copy(mean_bc[:], ps_bc[:])

            mean_g = mean_bc[:].rearrange("p (u c) -> p u c", u=1)

            # ---- per-group pipeline: center -> norms -> scale -> transpose
            # x first (its tiles gate every pass-1 matmul), then y groups in
            # quarter order. x's center/scale run on DVE so gpsimd (the
            # serial bottleneck of this phase) only carries the y groups.
            # Everything downstream of centering is bf16: transposes run at
            # 1 cyc/row and PSUM->SBUF copies hit the 2x packed DVE mode.
            ytc = [tmats.tile([P, CHUNK], MM_DT, tag=f"ytc{j}", name=f"ytc{j}")
                   for j in range(N // CHUNK)]     # y: [C, m] chunks
            xt = tmats.tile([P, ROWS], MM_DT)      # x: [C, n]

            GRP = 8
            std_all = singles.tile([P, NT], F32)
            # (src f32 view, centered bf16 view, tile base, is_x)
            groups = []
            for g in range(NXT // GRP):
                groups.append((xsp[:, g * GRP:(g + 1) * GRP, :],
                               xcb[:, g * GRP:(g + 1) * GRP, :],
                               NYT + g * GRP, True))
            for g in range(NYT // GRP):
                groups.append((ysp[:, g * GRP:(g + 1) * GRP, :],
                               ycb[:, g * GRP:(g + 1) * GRP, :],
                               g * GRP, False))

            # x is kept UNNORMALIZED: with s = ||xc_n|| and M-hat/S-tilde the
            # max/gram of the half-scaled S = G*inv_y, the identities
            #   c/s   = 1/(h*(s*(1+eps) - Mhat))        (exp scale)
            #   -M*c  = -Mhat*(c/s)                      (exp bias)
            # make the per-block chain identical in op count while x skips
            # its scale pass and reciprocal entirely.
            s1_x = singles.tile([P, NXT], F32)     # s*(1+eps) per x tile

            def stage_sub(src, cb, t0, is_x):
                eng = nc.vector if is_x else nc.gpsimd
                eng.tensor_sub(cb, src, mean_g.broadcast_to([P, GRP, C]))

            def stage_norm(src, cb, t0, is_x):
                sq = scratch.tile([P, GRP, C], F32, tag="sq")
                nc.scalar.activation(out=sq[:], in_=cb, func=AF.Square)
                nc.vector.reduce_sum(ns_all[:, t0:t0 + GRP], sq[:],
                                     axis=mybir.AxisListType.X)
                # norm = sqrt(ns); the reference's +1e-10 is far below
                # bf16 resolution of these O(10) norms
                nc.scalar.activation(std_all[:, t0:t0 + GRP],
                                     ns_all[:, t0:t0 + GRP], AF.Sqrt)
                if not is_x:
                    nc.vector.reciprocal(inv_all[:, t0:t0 + GRP],
                                         std_all[:, t0:t0 + GRP])

            def stage_scale(ps_tr_pool, src, cb, t0, is_x):
                if not is_x:
                    ig = inv_all[:, t0:t0 + GRP].rearrange(
                        "p (t u) -> p t u", u=1)
                    nc.gpsimd.tensor_mul(cb, cb,
                                         ig.broadcast_to([P, GRP, C]))
                for h in range(2):             # 2 batches of 4 tiles
                    pst = ps_tr_pool.tile([P, 4 * P], BF16, tag="pst")
                    for k in range(4):
                        nc.tensor.transpose(
                            pst[:, k * P:(k + 1) * P],
                            cb[:, 4 * h + k, :], identity_bf[:])
                    # split the PSUM->SBUF copies between DVE and the (idle
                    # during preproc) scalar engine
                    if is_x:
                        x0 = (t0 - NYT + 4 * h) * P
                        dst = xt[:, x0:x0 + 4 * P]
                    else:
                        dst = ytc[t0 // 4 + h][:]
                    if h == 0:
                        nc.vector.tensor_copy(dst, pst[:])
                    else:
                        nc.scalar.copy(dst, pst[:])

            # Issue all centers, then all norm chains, then all scales:
            # gpsimd runs its subs back to back, and the DVE casts (which
            # wait on PE transposes) queue after every norm reduce/recip so
            # they cannot head-block the y chain.
            with tc.tile_pool(name="ps_tr", bufs=4,
                              space=bass.MemorySpace.PSUM) as ps_tr_pool:
                for g in groups:
                    stage_sub(*g)
                for g in groups:
                    stage_norm(*g)
                nc.vector.tensor_scalar_mul(
                    s1_x[:], std_all[:, NYT:NYT + NXT], 1.0 + EPS_MIN)
                for g in groups:
                    stage_scale(ps_tr_pool, *g)

            # ---- main loop: two interleaved passes with recompute ----
            # Pass 1 (PE+DVE): matmul panels -> row-max, PSUM freed at max.
            # Pass 2 (PE+ACT): recompute the same panels -> exp in place with
            # accumulate. The second matmul is free (PE has slack) and the
            # recomputed S is bitwise identical. Passes run one block apart
            # in disjoint PSUM halves, so no engine waits on another's phase.
            cc_all = singles.tile([P, NRB], F32)   # 1/(H*(1+eps-Smax))
            nb_all = singles.tile([P, NRB], F32)   # -Smax*c
            r_all = singles.tile([P, NRB], F32)
            rq_all = singles.tile([P, NRB, NQ], F32)  # per-quarter exp sums

            def pass1(rb, pool):
                lhs = xt[:, rb * P:(rb + 1) * P]
                mxq = stat.tile([P, NQ], F32, tag="mxq", name="mxq")
                for q in range(NQ):
                    ps = pool.tile([P, QUART], F32, tag="p1", name="ps1")
                    for j in range(2):
                        nc.tensor.matmul(
                            ps[:, j * CHUNK:(j + 1) * CHUNK],
                            lhs, ytc[2 * q + j][:], start=True, stop=True)
                    nc.vector.reduce_max(mxq[:, q:q + 1], ps[:],
                                         axis=mybir.AxisListType.X)
                mx = stat.tile([P, 1], F32, tag="mx", name="mx")
                nc.vector.reduce_max(mx[:], mxq[:],
                                     axis=mybir.AxisListType.X)
                # scalar chain on gpsimd (idle during the main loop) except
                # the reciprocal, which only DVE has at full accuracy.
                # mx is Mhat (max of the x-unnormalized S); with
                # s1 = s*(1+eps) this yields cc = c/s and nb = -M*c, exactly
                # the scale/bias the exp needs for the true normalized S.
                hd = stat.tile([P, 1], F32, tag="hd", name="hd")
                nc.gpsimd.tensor_scalar(
                    out=hd[:], in0=mx[:], scalar1=s1_x[:, rb:rb + 1],
                    scalar2=-H_PARAM, op0=mybir.AluOpType.subtract,
                    op1=mybir.AluOpType.mult)
                nc.vector.reciprocal(cc_all[:, rb:rb + 1], hd[:])
                nc.gpsimd.tensor_scalar(
                    out=nb_all[:, rb:rb + 1], in0=mx[:],
                    scalar1=cc_all[:, rb:rb + 1], scalar2=-1.0,
                    op0=mybir.AluOpType.mult, op1=mybir.AluOpType.mult)

            def pass2(rb, pool):
                lhs = xt[:, rb * P:(rb + 1) * P]
                for q in range(NQ):
                    ps = pool.tile([P, QUART], F32, tag="p2", name="ps2")
                    for j in range(2):
                        nc.tensor.matmul(
                            ps[:, j * CHUNK:(j + 1) * CHUNK],
                            lhs, ytc[2 * q + j][:], start=True, stop=True)
                    nc.scalar.activation(
                        out=ps[:], in_=ps[:], func=AF.Exp,
                        bias=nb_all[:, rb:rb + 1], scale=cc_all[:, rb:rb + 1],
                        accum_out=rq_all[:, rb, q:q + 1])

            with (
                tc.tile_pool(name="ps_p1", bufs=2,
                             space=bass.MemorySpace.PSUM) as pool1,
                tc.tile_pool(name="ps_p2", bufs=2,
                             space=bass.MemorySpace.PSUM) as pool2,
            ):
                for rb in range(NRB + 1):
                    # issue pass2 first: its matmuls feed the ACT exp (the
                    # steady-state bottleneck), so they get PE queue priority
                    if rb >= 1:
                        pass2(rb - 1, pool2)
                    if rb < NRB:
                        pass1(rb, pool1)

            # ---- 1/r, reduce row contributions, write out ----
            nc.vector.reduce_sum(r_all[:], rq_all[:],
                                 axis=mybir.AxisListType.X)
            nc.vector.reciprocal(invr_all[:], r_all[:])
            part_sb = singles.tile([P, 1], F32)
            nc.vector.reduce_sum(part_sb[:], invr_all[:],
                                 axis=mybir.AxisListType.X)
            nc.scalar.dma_start(out=part[:], in_=part_sb[:])

    nc.compile()
    return nc


_NC_CACHE = None


def _get_nc():
    global _NC_CACHE
    if _NC_CACHE is None:
        _NC_CACHE = build_nc()
    return _NC_CACHE


def _in_maps(feature_x, feature_y):
    fx = np.ascontiguousarray(
        np.asarray(feature_x, dtype=np.float32).reshape(B, N, C))
    fy = np.ascontiguousarray(
        np.asarray(feature_y, dtype=np.float32).reshape(B, N, C))
    maps = []
    for core in range(8):
        b, h = divmod(core, 2)
        maps.append({
            "fx": np.ascontiguousarray(fx[b, h * ROWS:(h + 1) * ROWS, :]),
            "fy": fy[b],
        })
    return maps


def _combine(results):
    sums = [float(np.asarray(r["part"], dtype=np.float64).sum())
            for r in results]
    loss = np.empty(B, dtype=np.float64)
    for b in range(B):
        cx = (sums[2 * b] + sums[2 * b + 1]) / N
        loss[b] = -np.log(cx)
    return loss.astype(np.float32)


def kernel(feature_x, feature_y):
    nc = _get_nc()
    res = run_bass_kernel_spmd(nc, _in_maps(feature_x, feature_y),
                               core_ids=list(range(8)))
    return _combine(res.results)


def kernel_traced(feature_x, feature_y, **kwargs):
    """Like kernel() but with tracing; returns (loss, BassKernelResults)."""
    nc = _get_nc()
    res = run_bass_kernel_spmd(nc, _in_maps(feature_x, feature_y),
                               core_ids=list(range(8)), trace=True, **kwargs)
    return _combine(res.results), res



# revision 26
# speedup vs baseline: 1.0351x; 1.0081x over previous
"""Contextual loss kernel for Trainium2 (Bass/Tile), 8 NeuronCores.

Reference computation (per batch b, B=4, C=128, N=64*64=4096):
  mean_y[c] = spatial mean of feature_y
  fx,fy centered by mean_y; columns L2-normalized over channels
  S[n,m]    = <fxn[:,n], fyn[:,m]>           (cosine similarity)
  d = 1-S;  d_norm = d / (min_m d + 1e-3);  w = exp((1-d_norm)/h);  A = w/sum_m w
  CX[b] = mean_n max_m A;  loss = -log(CX)

Per-row identity used on device (with Smax = max_m S, c = 1/(h*(1-Smax+eps))):
  max_m A = 1 / sum_m exp(c*(S[m]-Smax))

Sharding: 8 cores = 4 batches x 2 row-halves. Each core gets its half of
feature_x's rows ([2048,128]) plus the full feature_y ([4096,128]) of its
batch, computes sum_rows 1/r locally; host combines and takes -log.
Row/column order within a core is permuted (contiguous-DMA layout); all
reductions over S are permutation-invariant so the result is unchanged.

Head: inputs land via two HWDGE queues with one large descriptor per
partition; the y mean accumulates per chunk as it arrives. Preprocessing
(center on gpsimd/DVE, square+sqrt on ACT, norm reduce+recip on DVE,
scale, bf16 PE transpose, 2x-mode PSUM->SBUF casts) is software-pipelined
per 8-tile group so the main loop starts as soon as the last y group lands.

Main loop per 128-row block (all bf16 matmuls, two passes, recompute):
pass 1 fills four 2-bank PSUM quarters and DVE row-maxes each (the row max
must complete before any exp: c = 1/(h*(1+eps-Smax)) scales the exponent);
a tiny chain (gpsimd+DVE) builds c and bias = -Smax*c; pass 2 recomputes
the same panels one block behind and ACT exp's each quarter in place with
accumulate (row sums). Recompute keeps PE streaming: PSUM (8 banks) cannot
hold two blocks, and every engine reading PSUM f32 is capped at 1
elem/cycle/lane, so S is cheaper to recompute than to stage elsewhere.
Steady state is ACT-bound (~5.3us/block): exp is 1 elem/cycle/lane at
1.2 GHz and only the scalar engine has exp.
"""

import numpy as np

import concourse.bacc as bacc
import concourse.bass as bass
import concourse.tile as tile
from concourse import masks, mybir
from concourse.bass_utils import run_bass_kernel_spmd

F32 = mybir.dt.float32
BF16 = mybir.dt.bfloat16
F32R = mybir.dt.float32r
AF = mybir.ActivationFunctionType

B = 4
C = 128
N = 4096          # spatial positions per batch
ROWS = N // 2     # rows of S per core (x-half)
P = 128           # partitions
NYT = N // P      # 32 y tiles
NXT = ROWS // P   # 16 x tiles
NT = NYT + NXT    # 48 tiles to preprocess
CHUNK = 512       # matmul free dim (one PSUM bank)
QUART = 1024      # columns per PSUM quarter (2 banks)
NQ = N // QUART   # 4 quarters per row block
NRB = ROWS // P   # 16 row blocks per core

H_PARAM = 0.1
EPS_MIN = 0.001
EPS_NORM = 1e-10

# bf16 matmul: 1 cyc/row at any PE p-state (f32r needs full 2.4GHz clock to
# hit 1 cyc/row; at the mid p-state it is no faster but the PE rarely ramps
# in this kernel). bf16 input quantization adds ~1e-3 loss error (tol 2e-2).
MM_DT = mybir.dt.bfloat16


def build_nc():
    nc = bacc.Bacc(None)
    fx = nc.declare_dram_parameter("fx", [ROWS, C], F32, isOutput=False)
    fy = nc.declare_dram_parameter("fy", [N, C], F32, isOutput=False)
    part = nc.declare_dram_parameter("part", [P, 1], F32, isOutput=True)

    # Partition p takes CONTIGUOUS spatial rows [k*p, k*(p+1)): one large
    # descriptor per partition per DMA instead of 512B strided lines. This
    # permutes S's rows/columns vs the reference, but every reduction over
    # them (row max, row sum, final sum over rows) is permutation-invariant.
    fy_t = fy.rearrange("(p i) c -> p i c", p=P)   # [128, 32, 128]
    fx_t = fx.rearrange("(p i) c -> p i c", p=P)   # [128, 16, 128]

    with tile.TileContext(nc) as tc:
        with (
            tc.tile_pool(name="singles", bufs=1) as singles,
            tc.tile_pool(name="raw", bufs=1) as raw,
            tc.tile_pool(name="tmats", bufs=1) as tmats,
            tc.tile_pool(name="stat", bufs=3) as stat,
            tc.tile_pool(name="scratch", bufs=2) as scratch,
        ):
            # ---- constants ----
            identity_bf = singles.tile([P, P], BF16)
            masks.make_identity(nc, identity_bf[:])
            ones_col = singles.tile([P, 1], F32)
            nc.vector.memset(ones_col[:], 1.0)
            ones_row = singles.tile([1, P], F32)
            nc.vector.memset(ones_row[:], 1.0)

            ns_all = singles.tile([P, NT], F32)     # squared norms per tile
            inv_all = singles.tile([P, NT], F32)    # 1/(norm+eps)
            invr_all = singles.tile([P, NRB], F32)  # per-rowblock 1/r
            mean_sb = singles.tile([1, C], F32)
            mean_bc = singles.tile([P, C], F32)
            colsum = singles.tile([P, C], F32)      # per-partition y colsums

            # ---- load inputs; y on the sync DGE queue, x on scalar's ----
            # Partial column sums per 4-tile chunk start as soon as each
            # chunk lands, so the mean is ready right after the last chunk.
            ysp = raw.tile([P, NYT, C], F32)   # y, spatial-major tiles
            xsp = raw.tile([P, NXT, C], F32)
            ycb = raw.tile([P, NYT, C], BF16)  # centered+normalized, bf16
            xcb = raw.tile([P, NXT, C], BF16)
            # y split across both HWDGE queues so the mean unblocks sooner;
            # x follows on the scalar queue (not needed until after mean).
            # colsum accumulates per chunk with CONTIGUOUS tensor_adds
            # (a strided reduce_sum over the tile axis runs at half rate and
            # held the mean - and with it every downstream center - hostage).
            for j in range(8):
                eng = nc.sync if j % 2 == 0 else nc.scalar
                eng.dma_start(
                    out=ysp[:, j * 4:(j + 1) * 4, :],
                    in_=fy_t[:, j * 4:(j + 1) * 4, :],
                )
                t4 = ysp[:, j * 4:(j + 1) * 4, :]
                mtmp = scratch.tile([P, 2, C], F32, tag="msum")
                nc.vector.tensor_add(mtmp[:], t4[:, 0:2, :], t4[:, 2:4, :])
                if j == 0:
                    nc.vector.tensor_add(colsum[:], mtmp[:, 0, :],
                                         mtmp[:, 1, :])
                else:
                    mtmp2 = scratch.tile([P, C], F32, tag="msum2")
                    nc.vector.tensor_add(mtmp2[:], mtmp[:, 0, :],
                                         mtmp[:, 1, :])
                    nc.vector.tensor_add(colsum[:], colsum[:], mtmp2[:])
            for j in range(2):
                nc.scalar.dma_start(
                    out=xsp[:, j * 8:(j + 1) * 8, :],
                    in_=fx_t[:, j * 8:(j + 1) * 8, :],
                )
            # PE: sum over partitions (ones^T @ colsum), scale by 1/N,
            # broadcast back over partitions via K=1 matmul
            with tc.tile_pool(name="ps_mean", bufs=1,
                              space=bass.MemorySpace.PSUM) as ps_mean_pool:
                ps_mean = ps_mean_pool.tile([1, C], F32)
                nc.tensor.matmul(ps_mean[:], ones_col[:], colsum[:],
                                 start=True, stop=True)
                nc.scalar.mul(mean_sb[:], ps_mean[:], 1.0 / N)
            with tc.tile_pool(name="ps_bc", bufs=1,
                              space=bass.MemorySpace.PSUM) as ps_bc_pool:
                ps_bc = ps_bc_pool.tile([P, C], F32)
                nc.tensor.matmul(ps_bc[:], ones_row[:], mean_sb[:],
                                 start=True, stop=True)
                nc.vector.tensor_copy(mean_bc[:], ps_bc[:])

            mean_g = mean_bc[:].rearrange("p (u c) -> p u c", u=1)

            # ---- per-group pipeline: center -> norms -> scale -> transpose
            # x first (its tiles gate every pass-1 matmul), then y groups in
            # quarter order. x's center/scale run on DVE so gpsimd (the
            # serial bottleneck of this phase) only carries the y groups.
            # Everything downstream of centering is bf16: transposes run at
            # 1 cyc/row and PSUM->SBUF copies hit the 2x packed DVE mode.
            ytc = [tmats.tile([P, CHUNK], MM_DT, tag=f"ytc{j}", name=f"ytc{j}")
                   for j in range(N // CHUNK)]     # y: [C, m] chunks
            xt = tmats.tile([P, ROWS], MM_DT)      # x: [C, n]

            GRP = 8
            std_all = singles.tile([P, NT], F32)
            # (src f32 view, centered bf16 view, tile base, is_x)
            groups = []
            for g in range(NXT // GRP):
                groups.append((xsp[:, g * GRP:(g + 1) * GRP, :],
                               xcb[:, g * GRP:(g + 1) * GRP, :],
                               NYT + g * GRP, True))
            for g in range(NYT // GRP):
                groups.append((ysp[:, g * GRP:(g + 1) * GRP, :],
                               ycb[:, g * GRP:(g + 1) * GRP, :],
                               g * GRP, False))

            # x is kept UNNORMALIZED: with s = ||xc_n|| and M-hat/S-tilde the
            # max/gram of the half-scaled S = G*inv_y, the identities
            #   c/s   = 1/(h*(s*(1+eps) - Mhat))        (exp scale)
            #   -M*c  = -Mhat*(c/s)                      (exp bias)
            # make the per-block chain identical in op count while x skips
            # its scale pass and reciprocal entirely.
            s1_x = singles.tile([P, NXT], F32)     # s*(1+eps) per x tile

            def stage_sub(src, cb, t0, is_x):
                eng = nc.vector if is_x else nc.gpsimd
                eng.tensor_sub(cb, src, mean_g.broadcast_to([P, GRP, C]))

            def stage_norm(src, cb, t0, is_x):
                sq = scratch.tile([P, GRP, C], F32, tag="sq")
                nc.scalar.activation(out=sq[:], in_=cb, func=AF.Square)
                nc.vector.reduce_sum(ns_all[:, t0:t0 + GRP], sq[:],
                                     axis=mybir.AxisListType.X)
                # norm = sqrt(ns); the reference's +1e-10 is far below
                # bf16 resolution of these O(10) norms
                nc.scalar.activation(std_all[:, t0:t0 + GRP],
                                     ns_all[:, t0:t0 + GRP], AF.Sqrt)
                if not is_x:
                    nc.vector.reciprocal(inv_all[:, t0:t0 + GRP],
                                         std_all[:, t0:t0 + GRP])

            def stage_scale(ps_tr_pool, src, cb, t0, is_x):
                if not is_x:
                    ig = inv_all[:, t0:t0 + GRP].rearrange(
                        "p (t u) -> p t u", u=1)
                    nc.gpsimd.tensor_mul(cb, cb,
                                         ig.broadcast_to([P, GRP, C]))
                for h in range(2):             # 2 batches of 4 tiles
                    pst = ps_tr_pool.tile([P, 4 * P], BF16, tag="pst")
                    for k in range(4):
                        nc.tensor.transpose(
                            pst[:, k * P:(k + 1) * P],
                            cb[:, 4 * h + k, :], identity_bf[:])
                    # split the PSUM->SBUF copies between DVE and the (idle
                    # during preproc) scalar engine
                    if is_x:
                        x0 = (t0 - NYT + 4 * h) * P
                        dst = xt[:, x0:x0 + 4 * P]
                    else:
                        dst = ytc[t0 // 4 + h][:]
                    if h == 0:
                        nc.vector.tensor_copy(dst, pst[:])
                    else:
                        nc.scalar.copy(dst, pst[:])

            # Issue all centers, then all norm chains, then all scales:
            # gpsimd runs its subs back to back, and the DVE casts (which
            # wait on PE transposes) queue after every norm reduce/recip so
            # they cannot head-block the y chain.
            with tc.tile_pool(name="ps_tr", bufs=4,
                              space=bass.MemorySpace.PSUM) as ps_tr_pool:
                for g in groups:
                    stage_sub(*g)
                for g in groups:
                    stage_norm(*g)
                nc.vector.tensor_scalar_mul(
                    s1_x[:], std_all[:, NYT:NYT + NXT], 1.0 + EPS_MIN)
                for g in groups:
                    stage_scale(ps_tr_pool, *g)

            # ---- main loop: two interleaved passes with recompute ----
            # Pass 1 (PE+DVE): matmul panels -> row-max, PSUM freed at max.
            # Pass 2 (PE+ACT): recompute the same panels -> exp in place with
            # accumulate. The second matmul is free (PE has slack) and the
            # recomputed S is bitwise identical. Passes run one block apart
            # in disjoint PSUM halves, so no engine waits on another's phase.
            cc_all = singles.tile([P, NRB], F32)   # 1/(H*(1+eps-Smax))
            nb_all = singles.tile([P, NRB], F32)   # -Smax*c
            r_all = singles.tile([P, NRB], F32)
            rq_all = singles.tile([P, NRB, NQ], F32)  # per-quarter exp sums

            def pass1(rb, pool):
                lhs = xt[:, rb * P:(rb + 1) * P]
                mxq = stat.tile([P, NQ], F32, tag="mxq", name="mxq")
                for q in range(NQ):
                    ps = pool.tile([P, QUART], F32, tag="p1", name="ps1")
                    for j in range(2):
                        nc.tensor.matmul(
                            ps[:, j * CHUNK:(j + 1) * CHUNK],
                            lhs, ytc[2 * q + j][:], start=True, stop=True)
                    nc.vector.reduce_max(mxq[:, q:q + 1], ps[:],
                                         axis=mybir.AxisListType.X)
                mx = stat.tile([P, 1], F32, tag="mx", name="mx")
                nc.vector.reduce_max(mx[:], mxq[:],
                                     axis=mybir.AxisListType.X)
                # scalar chain on gpsimd (idle during the main loop) except
                # the reciprocal, which only DVE has at full accuracy.
                # mx is Mhat (max of the x-unnormalized S); with
                # s1 = s*(1+eps) this yields cc = c/s and nb = -M*c, exactly
                # the scale/bias the exp needs for the true normalized S.
                hd = stat.tile([P, 1], F32, tag="hd", name="hd")
                nc.gpsimd.tensor_scalar(
                    out=hd[:], in0=mx[:], scalar1=s1_x[:, rb:rb + 1],
                    scalar2=-H_PARAM, op0=mybir.AluOpType.subtract,
                    op1=mybir.AluOpType.mult)
                nc.vector.reciprocal(cc_all[:, rb:rb + 1], hd[:])
                nc.gpsimd.tensor_scalar(
                    out=nb_all[:, rb:rb + 1], in0=mx[:],
                    scalar1=cc_all[:, rb:rb + 1], scalar2=-1.0,
                    op0=mybir.AluOpType.mult, op1=mybir.AluOpType.mult)

            def pass2(rb, pool):
                lhs = xt[:, rb * P:(rb + 1) * P]
                for q in range(NQ):
                    ps = pool.tile([P, QUART], F32, tag="p2", name="ps2")
                    for j in range(2):
                        nc.tensor.matmul(
                            ps[:, j * CHUNK:(j + 1) * CHUNK],
                            lhs, ytc[2 * q + j][:], start=True, stop=True)
                    nc.scalar.activation(
                        out=ps[:], in_=ps[:], func=AF.Exp,
                        bias=nb_all[:, rb:rb + 1], scale=cc_all[:, rb:rb + 1],
                        accum_out=rq_all[:, rb, q:q + 1])

            with (
                tc.tile_pool(name="ps_p1", bufs=2,
                             space=bass.MemorySpace.PSUM) as pool1,
                tc.tile_pool(name="ps_p2", bufs=2,
                             space=bass.MemorySpace.PSUM) as pool2,
            ):
                for rb in range(NRB + 1):
                    # issue pass2 first: its matmuls feed the ACT exp (the
                    # steady-state bottleneck), so they get PE queue priority
                    if rb >= 1:
                        pass2(rb - 1, pool2)
                    if rb < NRB:
                        pass1(rb, pool1)

            # ---- 1/r, reduce row contributions, write out ----
            nc.vector.reduce_sum(r_all[:], rq_all[:],
                                 axis=mybir.AxisListType.X)
            nc.vector.reciprocal(invr_all[:], r_all[:])
            part_sb = singles.tile([P, 1], F32)
            nc.vector.reduce_sum(part_sb[:], invr_all[:],
                                 axis=mybir.AxisListType.X)
            nc.scalar.dma_start(out=part[:], in_=part_sb[:])

    nc.compile()
    return nc


_NC_CACHE = None


def _get_nc():
    global _NC_CACHE
    if _NC_CACHE is None:
        _NC_CACHE = build_nc()
    return _NC_CACHE


def _in_maps(feature_x, feature_y):
    fx = np.ascontiguousarray(
        np.asarray(feature_x, dtype=np.float32).reshape(B, N, C))
    fy = np.ascontiguousarray(
        np.asarray(feature_y, dtype=np.float32).reshape(B, N, C))
    maps = []
    for core in range(8):
        b, h = divmod(core, 2)
        maps.append({
            "fx": np.ascontiguousarray(fx[b, h * ROWS:(h + 1) * ROWS, :]),
            "fy": fy[b],
        })
    return maps


def _combine(results):
    sums = [float(np.asarray(r["part"], dtype=np.float64).sum())
            for r in results]
    loss = np.empty(B, dtype=np.float64)
    for b in range(B):
        cx = (sums[2 * b] + sums[2 * b + 1]) / N
        loss[b] = -np.log(cx)
    return loss.astype(np.float32)


def kernel(feature_x, feature_y):
    nc = _get_nc()
    res = run_bass_kernel_spmd(nc, _in_maps(feature_x, feature_y),
                               core_ids=list(range(8)))
    return _combine(res.results)


def kernel_traced(feature_x, feature_y, **kwargs):
    """Like kernel() but with tracing; returns (loss, BassKernelResults)."""
    nc = _get_nc()
    res = run_bass_kernel_spmd(nc, _in_maps(feature_x, feature_y),
                               core_ids=list(range(8)), trace=True, **kwargs)
    return _combine(res.results), res

